# revision 33
# baseline (speedup 1.0000x reference)
"""Multi-head attention block (b=4, n=2048, d=256, h=8) on 8 TRN2 NeuronCores.

Sharding: core c handles (batch bi=c//2, query-half qh=c%2): it computes
K/V for the full sequence of its batch and Q for its 1024-row query half,
producing 1024 complete rows of the final output (host concatenates and
adds b_out; no cross-core reduction).

Design (matmul operands fp16; PSUM fp32). Attention runs per
(head-group of 4, q-chunk of 256) over 16 k-tiles of 128 keys:

  - Scores: TWO matmuls per k-tile (a matmul's output must fit one
    PSUM bank = 512 fp32): lhsT = 4-heads-stacked kT [128,128], rhs =
    qT_q[hg][qc] [128, (4 heads, 256 q)] halves; each (head j, q)
    column is zero-padded outside rows 32j..32j+32 so the stacked kT
    is masked per column. S psum [128, 2, 512].
  - exp SPLITS engines per k-tile: ACT exact Exp on pair-0's 512 cols
    (933ns for 1024 cols was the old pipeline's pacer at ~1005ns/kt;
    512 cols = 720ns), DVE Schraudolph int16 exp on pair-1's 512 cols
    (i16 = rint(dots*SCALE*1024/ln2 + B); bitcast fp16 ~ exp, ~2.7%
    sawtooth, C tuned zero-mean; the 2^-C factor cancels per-pair in
    the normalize). Each pair's AV rhs comes from ONE engine's tile so
    no extra matmul segments (extra LDWEIGHTS would eat the win).
    Chunk (0,0) keeps all-exp-on-ACT: its DVE carries the woven v/kT
    casts and the chunk is PE-bound on woven projections anyway.
  - AV: TWO matmuls per k-tile (pairs of heads): lhsT = [v_h(32) |
    v_h'(32) | ones(64)] [128, 128]. Rows 64..127 of the psum all
    accumulate the SAME softmax denominators of BOTH pair members
    (ones columns x probs); member e's values sit 32-aligned at rows
    32e..32e+31. Off-diagonal blocks are dead values (PE cost is
    columns only). The 64 replicated den rows exist so the normalize
    needs NO partition-broadcast: two quadrant-aligned 32-partition
    DVE reciprocals (a[64:96]->bc[0:32], a[96:128]->bc[32:64]) give
    1/den on exactly the base partitions the muls need (in0/in1 of a
    Pool/DVE tensor op must share their base partition). This replaced
    a denb-DMA -> reciprocal -> DRAM -> broadcast-read-DMA chain whose
    ~5us latency + queue coupling stalled the whole pipeline ~2.5-4us
    at every hg=1 chunk boundary (cadence 1300-1442 vs 1000 ns/kt).
  - AV lags S by TWO k-tiles so the PE never waits on exp latency.
  - Projections (Q^T padded, kT stacked, [v|1]) are WOVEN into the
    first attention iterations >=2 k-tiles ahead of use.
  - Boundary: a-copy av2->SBUF is split DVE (bank 0) / ACT-Copy
    (bank 1) so neither engine eats the full 1.2us; deferred stages of
    chunk g drain at fixed k-tile slots of chunk g+1 (den DMA kt1,
    reciprocal+broadcast DMAs kt4 -- after the den DMA's ~2.5us round
    trip has landed, so the reciprocal never head-of-line blocks the
    DVE hack stream -- Pool muls kt10, outproj kt14). The LAST chunk
    has nothing to hide the 3-serial-DMA broadcast behind, so its
    normalize broadcasts on-chip instead: 8 PE transposes gather the
    den row to [128,8], exact DVE reciprocal, 8 PE transposes back to
    a [1,1024] row, and ones-column matmuls replicate it to 64
    partitions (~9.6us tail vs ~23.5 with the DMA chain).
  - Startup: all 17 input DMAs + 24 memsets are spread across five
    engine queues first-needed-first (a dma_start costs ~650ns of
    issue time on ANY engine; the old 2-queue issue + 8us of qT
    memsets on gpsimd pushed the first exp to 25.9us).
  - PSUM: S 2x2 banks + av2 2 + proj/outproj 2 = 8 banks.

Host: uploads fp16 inputs (halves DMA), adds b_out and the exact v-bias
image b_v @ w_out (softmax rows sum to 1 => attn@(v+b_v) = attn@v+b_v);
k-bias drops (adds a per-query constant, cancels in softmax).
"""
import numpy as np

import concourse.bacc as bacc
import concourse.bass as bass
import concourse.masks as masks
import concourse.mybir as mybir
import concourse.tile as tile
from concourse.bass_utils import run_bass_kernel_spmd

F32 = mybir.dt.float32
F16 = mybir.dt.float16
I16 = mybir.dt.int16
Exp = mybir.ActivationFunctionType.Exp
Copy = mybir.ActivationFunctionType.Copy
MUL = mybir.AluOpType.mult
ADD = mybir.AluOpType.add

B, N, D = 4, 2048, 256
H, DH = 8, 32
NQ = N // 2            # per-core query rows
SCALE = D ** -0.5      # 0.0625
NKT = N // 128         # 16 k-tiles
QC = 256               # q-chunk
NQC = NQ // QC         # 4 q-chunks per core

LN2 = float(np.log(2.0))
HACK_C = 0.0573        # zero-mean shift for the Schraudolph sawtooth
HACK_A = SCALE * 1024.0 / LN2
HACK_B = 15.0 * 1024.0 - HACK_C * 1024.0
VST_W = 65             # [v_h | v_h' | ones] per pair block

_BUILD_CACHE = {}


def build():
    if "nc" in _BUILD_CACHE:
        return _BUILD_CACHE["nc"]
    nc = bacc.Bacc()

    xT_d = nc.dram_tensor("xT", [D, N], F16, kind="ExternalInput")
    xqT_d = nc.dram_tensor("xqT", [D, NQ], F16, kind="ExternalInput")
    w_d = nc.dram_tensor("w_qkv", [D, 3 * D], F16, kind="ExternalInput")
    b_d = nc.dram_tensor("b_qkv", [1, 3 * D], F16, kind="ExternalInput")
    wo_d = nc.dram_tensor("w_out", [D, D], F16, kind="ExternalInput")
    out_d = nc.dram_tensor("out", [NQ, D], F32, kind="ExternalOutput")
    recip_dram = nc.dram_tensor("recip_scratch", [2, NQC, 1024], F32)

    with tile.TileContext(nc) as tc:
        with (
            tc.tile_pool(name="persist", bufs=1) as persist,
            tc.tile_pool(name="probs", bufs=4) as prpool,
            tc.tile_pool(name="hackt", bufs=4) as tpool,
            tc.tile_pool(name="avsb", bufs=2) as avsb_pool,
            tc.tile_pool(name="norm", bufs=2) as norm_pool,
            tc.tile_pool(name="outsb", bufs=3) as out_pool,
            tc.tile_pool(name="kqps", bufs=1, space="PSUM") as kqps,
            tc.tile_pool(name="sps0", bufs=2, space="PSUM") as sps0,
            tc.tile_pool(name="sps1", bufs=3, space="PSUM") as sps1,
            tc.tile_pool(name="avps", bufs=1, space="PSUM") as avps,
        ):
            # ---- persistent tiles ----
            ones = persist.tile([1, 512], F16, name="ones")
            w_sb = [persist.tile([128, 3 * D], F16, name=f"w{d2}") for d2 in range(2)]
            b_sb = persist.tile([1, 3 * D], F16, name="b_sb")
            xT_sb = [[persist.tile([128, 512], F16, name=f"xT{d2}_{c}")
                      for c in range(4)] for d2 in range(2)]
            xqT_sb = [[persist.tile([128, 512], F16, name=f"xq{d2}_{c}")
                       for c in range(2)] for d2 in range(2)]
            wo_sb = [persist.tile([128, D], F16, name=f"wo{g}") for g in range(2)]
            kT_c = [[persist.tile([128, 512], F16, name=f"kT{g}_{c}")
                     for c in range(4)] for g in range(2)]
            # per-(hg,qc) padded q: column (j, q) nonzero only rows 32j..32j+32
            qT_q = [[persist.tile([128, 4, QC], F16, name=f"qTq{g}_{c}")
                     for c in range(NQC)] for g in range(2)]
            # per k-tile: 4 head-pairs x [v_h(32) | v_h'(32) | ones(64)]
            v_st = [persist.tile([128, 4 * VST_W], F16, name=f"vst{s}")
                    for s in range(NKT)]
            outT_c = [[persist.tile([128, 256], F16, name=f"outT{g}_{c}")
                       for c in range(NQC)] for g in range(2)]

            def vst_ones(eng, s):
                eng.memset(
                    v_st[s].rearrange("p (pp s) -> p pp s", s=VST_W)[:, :, 64:VST_W],
                    1.0)

            # ---- startup: the 4 critical DMAs (w halves + xq chunk 0 --
            # the first projection's inputs) are the FIRST instruction of 4
            # different queues so their hardware DMA stream is not queued
            # behind the 1MB of xT. dma_start costs ~650ns of issue time on
            # any engine; a [128,1024]-f16 memset ~0.9us.
            nc.gpsimd.dma_start(out=w_sb[0], in_=w_d[0:128, :])
            nc.scalar.dma_start(out=xqT_sb[0][0], in_=xqT_d[0:128, 0:512])
            nc.sync.dma_start(out=xT_sb[0][0], in_=xT_d[0:128, 0:512])
            nc.sync.dma_start(out=xT_sb[1][0], in_=xT_d[128:256, 0:512])
            nc.sync.dma_start(out=w_sb[1], in_=w_d[128:256, :])
            nc.sync.dma_start(out=xqT_sb[1][0], in_=xqT_d[128:256, 0:512])
            nc.gpsimd.dma_start(out=b_sb, in_=b_d[:, :])
            # vector: first-chunk memsets (vst ones-cols are ~60ns each)
            nc.vector.memset(ones, 1.0)
            nc.vector.memset(qT_q[0][0], 0.0)
            for s in range(NKT):
                vst_ones(nc.vector, s)
            # scalar: q-half-1 x (needed at weave kt9 of chunk 0)
            for d2 in range(2):
                nc.scalar.dma_start(out=xqT_sb[d2][1],
                                    in_=xqT_d[128 * d2:128 * (d2 + 1), 512:1024])
            # sync: full-seq x chunks in c order, then w_out
            for c in range(1, 4):
                for d2 in range(2):
                    nc.sync.dma_start(
                        out=xT_sb[d2][c],
                        in_=xT_d[128 * d2:128 * (d2 + 1), 512 * c:512 * (c + 1)])
            for g in range(2):
                nc.sync.dma_start(out=wo_sb[g], in_=wo_d[128 * g:128 * (g + 1), :])
            # gpsimd: remaining qT pads in first-needed order
            for hg, t in ((0, 1), (0, 2), (0, 3), (1, 0), (1, 1), (1, 2), (1, 3)):
                nc.gpsimd.memset(qT_q[hg][t], 0.0)
            # tail-only: identity + ones for the PE-transpose broadcast
            # (matmul asserts lhsT.base_partition == rhs.base_partition, so
            # the ones operands live on the partitions of their co-operand)
            onesf32 = persist.tile([65, 1], F32, name="onesf32")
            nc.vector.memset(onesf32, 1.0)
            ident128 = persist.tile([128, 128], F32, name="ident128")
            masks.make_identity(nc, ident128[:, :])

            # unit psum->SBUF copies all on DVE: ACT runs the exact-exp half
            # (the critical chain) and any copy in its queue delays it
            def copy(out, in_):
                nc.vector.tensor_copy(out=out, in_=in_)

            # ---- projection units (woven into the attention stream).
            # pool/tag: the prefix instances borrow the then-idle S pools so
            # they don't serialize through the 1-deep kq ring; split_copy
            # halves a qT unit's 3.6us copy burst across DVE+ACT (ACT takes
            # the base-0/64 blocks; base 96 is only proven on DVE)
            def qT_unit(hg, c, pool=None, tag="kq", split_copy=False):
                """q^T for head-group hg, 512 q columns (q-chunks 2c, 2c+1)."""
                p = (pool or kqps).tile([128, 512], F32, tag=tag,
                                        name=f"kqq_{hg}_{c}")
                for d2 in range(2):
                    nc.tensor.matmul(
                        p[:, :], w_sb[d2][:, 128 * hg:128 * (hg + 1)],
                        xqT_sb[d2][c],
                        start=(d2 == 0), stop=False)
                nc.tensor.matmul(
                    p[:, :], b_sb[:, 128 * hg:128 * (hg + 1)], ones[:, :],
                    start=False, stop=True)
                for j in range(4):
                    for half in range(2):
                        dst = qT_q[hg][2 * c + half][32 * j:32 * (j + 1), j, :]
                        src = p[32 * j:32 * (j + 1), 256 * half:256 * (half + 1)]
                        if split_copy and j in (0, 2):
                            nc.scalar.activation(out=dst, in_=src, func=Copy)
                        else:
                            copy(dst, src)

            def kT_unit(hg, c, act_copy=False):
                """k^T for head-group hg, seq chunk c (512 wide)."""
                p = kqps.tile([128, 512], F32, tag="kq", name=f"kqk_{hg}_{c}")
                for d2 in range(2):
                    nc.tensor.matmul(
                        p[:, :], w_sb[d2][:, D + 128 * hg:D + 128 * (hg + 1)],
                        xT_sb[d2][c],
                        start=(d2 == 0), stop=(d2 == 1))
                if act_copy:
                    nc.scalar.activation(out=kT_c[hg][c][:, 0:256],
                                         in_=p[:, 0:256], func=Copy)
                    nc.scalar.activation(out=kT_c[hg][c][:, 256:512],
                                         in_=p[:, 256:512], func=Copy)
                else:
                    copy(kT_c[hg][c][:, :], p[:, :])

            def v_unit(st, pool=None, tag="kq"):
                """v rows for seq tile st (128 wide), all 8 heads."""
                p = (pool or kqps).tile([128, D], F32, tag=tag,
                                        name=f"vv_{st}")
                for d2 in range(2):
                    nc.tensor.matmul(
                        p[:, :], xT_sb[d2][st // 4][:, 128 * (st % 4):128 * (st % 4 + 1)],
                        w_sb[d2][:, 2 * D:3 * D],
                        start=(d2 == 0), stop=(d2 == 1))
                copy(v_st[st].rearrange("p (pp s) -> p pp s", s=VST_W)[:, :, 0:64],
                     p.rearrange("p (pp c) -> p pp c", pp=4))

            # weave schedule: units emitted >=2 k-tiles before first use
            weave = {}
            weave[(0, 0, 0)] = [lambda: v_unit(2)]
            weave[(0, 0, 1)] = [lambda: v_unit(3), lambda: kT_unit(0, 1)]
            for st in range(4, NKT):
                weave.setdefault((0, 0, st - 2), []).append(
                    lambda st=st: v_unit(st))
            weave.setdefault((0, 0, 3), []).append(lambda: kT_unit(0, 2))
            weave.setdefault((0, 0, 7), []).append(lambda: kT_unit(0, 3))
            weave.setdefault((0, 0, 9), []).append(
                lambda: qT_unit(0, 1, split_copy=True))
            weave[(0, 1, 2)] = [lambda: kT_unit(1, 0)]
            weave[(0, 1, 5)] = [lambda: kT_unit(1, 1)]
            weave[(0, 2, 0)] = [lambda: kT_unit(1, 2)]
            weave[(0, 2, 3)] = [lambda: kT_unit(1, 3)]
            weave[(0, 2, 6)] = [lambda: qT_unit(1, 1, split_copy=True)]

            # prefix: just enough for (hg0, qc0..1) k-tiles 0..3. qT(1,0)
            # also lives here (hidden under the ~17us xT DMA arrival; woven
            # into chunk 0's tail its copy burst stalled the PE ~2.6us at
            # the boundary). Distinct psum pools + the ACT-side copies keep
            # the units from serializing through the 1-deep kq ring: S(0)
            # only waits on qT(0,0)'s copies and kT(0,0)'s ACT copy.
            qT_unit(0, 0)
            qT_unit(1, 0, pool=sps1, tag="S1", split_copy=True)
            kT_unit(0, 0, act_copy=True)
            v_unit(0, pool=sps0, tag="S0")
            v_unit(1, pool=sps1, tag="S1")

            # ---- attention ----
            def emit_outproj(qc):
                for qt in (2 * qc, 2 * qc + 1):
                    po = kqps.tile([128, D], F32, tag="kq", name=f"po{qt}")
                    for g in range(2):
                        nc.tensor.matmul(
                            po[:, :],
                            outT_c[g][qt // 2][:, 128 * (qt % 2):128 * (qt % 2 + 1)],
                            wo_sb[g][:, :],
                            start=(g == 0), stop=(g == 1))
                    o = out_pool.tile([128, D], F32, tag="o", name=f"o{qt}")
                    nc.vector.tensor_copy(o, po[:, :])
                    nc.sync.dma_start(out=out_d[128 * qt:128 * (qt + 1), :],
                                      in_=o)

            def norm_mul(hg, qc, a, bc, e, p, eng):
                j = 2 * p + e
                eng.tensor_mul(
                    outT_c[hg][qc][32 * j:32 * (j + 1), :],
                    a[32 * e:32 * e + 32, p, 256 * e:256 * (e + 1)],
                    bc[32 * e:32 * e + 32, p, 256 * e:256 * (e + 1)])

            def norm_den_dma(hg, qc, a):
                """den row -> [128,8]: 8 elems/lane (DVE reciprocal is
                ~6.4ns/elem/lane, so wide layouts are 30x slower)."""
                denb = norm_pool.tile([128, 8], F32, tag="denb",
                                      name=f"denb{hg}_{qc}")
                nc.gpsimd.dma_start(out=denb, in_=a[64:65, :, :])
                return denb

            def norm_recip_bcast(hg, qc, denb):
                """exact reciprocal -> DRAM -> 64-row broadcast read.
                Drained 3 k-tiles after the den DMA so the reciprocal never
                sits at the head of the DVE queue waiting for the DMA round
                trip (that wait stalled the whole hack stream ~2.5us at
                chunk boundaries). DMAs on gpsimd: blocking there while a
                cross-engine dep lands is harmless."""
                recb = norm_pool.tile([128, 8], F32, tag="recb",
                                      name=f"recb{hg}_{qc}")
                nc.vector.reciprocal(recb, denb)
                nc.gpsimd.dma_start(out=recip_dram[hg, qc, :], in_=recb)
                # 64 partitions so each mul's two SBUF inputs share a base
                # partition (in0 at 32e must equal in1's base)
                bc = norm_pool.tile([64, 2, 512], F32, tag="bc",
                                    name=f"bc_{hg}_{qc}")
                row = recip_dram[hg, qc, :]
                nc.gpsimd.dma_start(
                    out=bc,
                    in_=bass.AP(tensor=row.tensor, offset=row.offset,
                                ap=[[0, 64], row.ap[-1]]))
                return bc

            # deferred stages of chunk g, drained at fixed k-tile slots of
            # chunk g+1 -- late enough that every dependency has landed.
            pending = []  # (due_g, slot_kt, fn)

            def drain(g, kt):
                for item in [it for it in pending
                             if it[0] == g and it[1] == kt]:
                    pending.remove(item)
                    item[2]()

            for hg in range(2):
                for qc in range(NQC):
                    av2 = avps.tile([65, 2, 512], F32, tag="av",
                                    name=f"av_{hg}_{qc}")

                    def emit_av(segs, kt, hg=hg, av2=av2):
                        for p in range(2):
                            pp = 2 * hg + p
                            for ofs, width, rhs in segs[p]:
                                nc.tensor.matmul(
                                    av2[:, p, ofs:ofs + width],
                                    v_st[kt][:, VST_W * pp:VST_W * pp + VST_W],
                                    rhs,
                                    start=(kt == 0 and ofs == 0),
                                    stop=(kt == NKT - 1))

                    g = 4 * hg + qc
                    hist = {}
                    for kt in range(NKT):
                        for u in weave.get((hg, qc, kt), ()):
                            u()
                        drain(g, kt)
                        # independent single-bank S tiles per pair: the ACT
                        # ring (pair 0) and the DVE ring (pair 1) release
                        # their PSUM slots independently, and the DVE ring is
                        # 3 deep -- the S(kt)->exp(kt)->S(kt+2) recurrence
                        # through a 2-deep shared tile plus two cross-engine
                        # semaphore hops was the steady-state pacer
                        # (~720ns waits on one S matmul every k-tile).
                        S0 = sps0.tile([128, 512], F32, tag="S0",
                                       name=f"S0_{hg}_{qc}_{kt}")
                        S1 = sps1.tile([128, 512], F32, tag="S1",
                                       name=f"S1_{hg}_{qc}_{kt}")
                        for p, St in ((0, S0), (1, S1)):
                            nc.tensor.matmul(
                                St[:, :],
                                kT_c[hg][kt // 4][:, 128 * (kt % 4):128 * (kt % 4 + 1)],
                                qT_q[hg][qc].rearrange("p a b -> p (a b)")[:, 512 * p:512 * (p + 1)],
                                start=True, stop=True)
                        pr = prpool.tile([128, 512], F16, tag="pr",
                                         name=f"pr_{hg}_{qc}_{kt}")
                        nc.scalar.activation(out=pr, in_=S0,
                                             func=Exp, scale=SCALE)
                        t = tpool.tile([128, 512], I16, tag="t",
                                       name=f"t_{hg}_{qc}_{kt}")
                        nc.vector.tensor_scalar(
                            out=t, in0=S1,
                            scalar1=HACK_A, scalar2=HACK_B,
                            op0=MUL, op1=ADD)
                        tv = t.bitcast(F16)
                        segs = ([(0, 512, pr[:, :])],
                                [(0, 512, tv[:, :])])
                        hist[kt] = segs
                        if kt >= 2:
                            emit_av(hist.pop(kt - 2), kt - 2)
                    emit_av(hist.pop(NKT - 2), NKT - 2)
                    emit_av(hist.pop(NKT - 1), NKT - 1)

                    # boundary a-copy split by bank: ~720ns on each engine
                    # (a [1,1024] den-row-only copy is 1-lane on DVE and
                    # measured 1.2us -- worse). Frees the av psum by ~kt1.5
                    # of the next chunk (its AV(0) issues at kt2).
                    a = avsb_pool.tile([65, 2, 512], F32, tag="avsb",
                                       name=f"avsb_{hg}_{qc}")
                    nc.vector.tensor_copy(a[:, 0, :], av2[:, 0, :])
                    nc.scalar.activation(out=a[:, 1, :], in_=av2[:, 1, :],
                                         func=Copy)
                    last = (hg == 1 and qc == NQC - 1)
                    if last:
                        for item in sorted(pending, key=lambda it: (it[0], it[1])):
                            pending.remove(item)
                            item[2]()
                        # tail: all-on-chip broadcast via PE transposes --
                        # the 3-serial-DMA chain costs ~9us of mostly fixed
                        # DMA latency with nothing left to hide it behind.
                        af = a.rearrange("p a b -> p (a b)")
                        dt = kqps.tile([128, 8], F32, tag="kq", name="dt")
                        for c8 in range(8):
                            nc.tensor.transpose(
                                dt[:, c8:c8 + 1],
                                af[64:65, 128 * c8:128 * (c8 + 1)],
                                onesf32[64:65, 0:1])
                        recb = norm_pool.tile([128, 8], F32, tag="recb",
                                              name="recb_t")
                        nc.vector.reciprocal(recb, dt)
                        # transpose back column-by-column: engine/matmul APs
                        # may only base at partition 0/32/64, so the recip
                        # row must land on partition 0 before broadcasting
                        rps = [sps0.tile([1, 512], F32, tag="S0", name="rps0"),
                               sps1.tile([1, 512], F32, tag="S1", name="rps1")]
                        for c8 in range(8):
                            nc.tensor.transpose(
                                rps[c8 // 4][0:1, 128 * (c8 % 4):128 * (c8 % 4 + 1)],
                                recb[:, c8:c8 + 1], ident128[:, :])
                        rT16 = norm_pool.tile([1, 1024], F16, tag="rt16",
                                              name="rT16")
                        nc.vector.tensor_copy(rT16[0:1, 0:512], rps[0])
                        nc.scalar.activation(out=rT16[0:1, 512:1024],
                                             in_=rps[1], func=Copy)
                        for p, pool in ((0, sps0), (1, sps1)):
                            ps = pool.tile([64, 512], F32,
                                           tag=("S0" if p == 0 else "S1"),
                                           name=f"bcps{p}")
                            for k in range(4):
                                nc.tensor.matmul(
                                    ps[:, 128 * k:128 * (k + 1)],
                                    ones[0:1, 0:64],
                                    rT16[0:1, 128 * (4 * p + k):128 * (4 * p + k + 1)],
                                    start=True, stop=True)
                            sb = norm_pool.tile([64, 512], F32, tag="bcsb",
                                                name=f"bcsb{p}")
                            if p == 0:
                                nc.scalar.activation(out=sb, in_=ps,
                                                     func=Copy)
                            else:
                                nc.vector.tensor_copy(sb, ps)
                            for e in range(2):
                                j = 2 * p + e
                                eng = nc.gpsimd if e == 0 else nc.vector
                                eng.tensor_mul(
                                    outT_c[hg][qc][32 * j:32 * (j + 1), :],
                                    a[32 * e:32 * e + 32, p,
                                      256 * e:256 * (e + 1)],
                                    sb[32 * e:32 * e + 32,
                                       256 * e:256 * (e + 1)])
                        emit_outproj(qc)
                    else:
                        def stage1(hg=hg, qc=qc, a=a, g=g):
                            denb = norm_den_dma(hg, qc, a)

                            def stage2(hg=hg, qc=qc, a=a, denb=denb, g=g):
                                bc = norm_recip_bcast(hg, qc, denb)

                                def muls(hg=hg, qc=qc, a=a, bc=bc):
                                    # Pool: it idle-blocks on deps
                                    # harmlessly; its library loads once
                                    for e in range(2):
                                        for p in range(2):
                                            norm_mul(hg, qc, a, bc, e, p,
                                                     nc.gpsimd)
                                pending.append((g + 1, 12, muls))
                            pending.append((g + 1, 5, stage2))
                        pending.append((g + 1, 1, stage1))
                        if hg == 1:
                            # (g+2, 4): the muls finish ~kt15.5 of chunk g+1
                            # (each DMA flight is ~3us, so the bc lands
                            # ~kt11.5); an outproj drained at (g+1, 14)
                            # head-of-line blocked the PE ~2.3us waiting on
                            # them. qc2's outproj (due g=8) falls through to
                            # the tail drain, where the PE is idle anyway.
                            def stage3(qc=qc):
                                emit_outproj(qc)
                            pending.append((g + 2, 4, stage3))

    nc.compile()
    _BUILD_CACHE["nc"] = nc
    return nc


def _run(x, w_qkv, b_qkv, w_out, trace=False):
    nc = build()
    x16 = np.asarray(x, np.float16)
    w16 = np.ascontiguousarray(np.asarray(w_qkv, np.float16))
    b16 = np.ascontiguousarray(np.asarray(b_qkv, np.float16).reshape(1, 3 * D))
    wo16 = np.ascontiguousarray(np.asarray(w_out, np.float16))
    in_maps = []
    for c in range(8):
        bi, qh = c // 2, c % 2
        in_maps.append({
            "xT": np.ascontiguousarray(x16[bi].T),
            "xqT": np.ascontiguousarray(x16[bi, NQ * qh:NQ * (qh + 1)].T),
            "w_qkv": w16,
            "b_qkv": b16,
            "w_out": wo16,
        })
    res = run_bass_kernel_spmd(nc, in_maps, core_ids=list(range(8)), trace=trace)
    out = np.empty((B, N, D), dtype=np.float32)
    for c in range(8):
        bi, qh = c // 2, c % 2
        out[bi, NQ * qh:NQ * (qh + 1)] = res.results[c]["out"]
    # v-bias correction (exact): attn@(v+b_v) = attn@v + b_v, so the device
    # omits b_v and the host adds its image through the output projection.
    bv = np.asarray(b_qkv, np.float32).reshape(-1)[2 * D:3 * D]
    out += (bv @ np.asarray(w_out, np.float32))[None, None, :]
    return out, res


def kernel(x, w_qkv, b_qkv, w_out, b_out):
    x = np.asarray(x, dtype=np.float32)
    out, _ = _run(x, np.asarray(w_qkv, np.float32), np.asarray(b_qkv, np.float32),
                  np.asarray(w_out, np.float32))
    return out + np.asarray(b_out, np.float32)[None, None, :]
